# revision 2
# baseline (speedup 1.0000x reference)
"""Trainium2 Bass kernel for nn_MetaMixer_6717328851330.

Computation (see reference):
    p = x @ W_in.T ; h, gate = split(p) ; gate = silu(gate)
    h = causal_grouped_conv1d(h) + b_conv ; h = h * gate       (residual)
    hn = layernorm_I(h) ; m = silu(hn @ W_fc.T + b_fc) @ W_cp.T + b_cp
    y = (m + residual) @ W_out.T

Strategy: pure data-parallel over the 8192 tokens (B*L), 1024 tokens per
core, no collectives. The causal-conv left halo (3 tokens) is handled by
shipping the previous chunk's last 3 h-columns to each core (host
precomputes them) and carrying the segment boundary on-chip.

On-core layout: activations live as [channel, token] tiles so every matmul
in the chain contracts along SBUF partitions with no transposes anywhere
(host pre-transposes x and all weights). The whole matmul pipeline runs in
bf16 (weights and activations; fp32 PSUM accumulate) — bf16 streams at the
same 1 row/cycle as fp32r but halves LDWEIGHTS and all DMA traffic.
Biases are kept off the PE: conv bias rides the fused DVE
(psum+b)*gate op, the c_proj bias is folded into a final per-channel
output bias (W_out @ b_cproj), and the fc bias rides the silu activation.
Each core processes its 1024 tokens in two 512-token segments so all
intermediates stay resident in SBUF; weights stream from HBM per segment.
"""
import sys

sys.path.insert(0, "/opt/trn_rl_repo")
import numpy as np

NCORES = 8
B, L, H, I, G, CK = 2, 4096, 1024, 2048, 8, 4
T = (B * L) // NCORES          # tokens per core
S = 512                        # token segment (= psum bank free dim)
NSEG = T // S
HK = H // 128                  # 8  k-chunks over hidden
IK = I // 128                  # 16 k-chunks over intermediate
QC = NCORES // B               # seq chunks per batch
EPS = 1e-5
WBUFS = 3                      # streamed weight blocks [128,4096] bf16

_CACHE = {}


def _build():
    import concourse.bacc as bacc
    import concourse.mybir as mybir
    import concourse.tile as tile
    from concourse.alu_op_type import AluOpType

    f32 = mybir.dt.float32
    bf16 = mybir.dt.bfloat16
    AF = mybir.ActivationFunctionType
    MUL, ADD, SUB = AluOpType.mult, AluOpType.add, AluOpType.subtract

    nc = bacc.Bacc(None, target_bir_lowering=False)

    xT = nc.dram_tensor("xT", [128, NSEG * HK * S], bf16, kind="ExternalInput")
    win = nc.dram_tensor("win", [128, HK * 2 * I], bf16, kind="ExternalInput")
    cw = nc.dram_tensor("cw", [128, G * 2 * CK * 256], bf16, kind="ExternalInput")
    fcw = nc.dram_tensor("fcw", [128, I * H // 128], bf16, kind="ExternalInput")
    cpw = nc.dram_tensor("cpw", [128, H * I // 128], bf16, kind="ExternalInput")
    outw = nc.dram_tensor("outw", [128, I * H // 128], bf16, kind="ExternalInput")
    cbcol = nc.dram_tensor("cbcol", [128, IK], f32, kind="ExternalInput")
    youb = nc.dram_tensor("youb", [128, HK], f32, kind="ExternalInput")
    sfc = nc.dram_tensor("sfc", [1, H], bf16, kind="ExternalInput")
    haloh = nc.dram_tensor("haloh", [128, IK * 4], bf16, kind="ExternalInput")
    fcb = nc.dram_tensor("fcb", [128, HK], f32, kind="ExternalInput")
    yT = nc.dram_tensor("yT", [H, T], f32, kind="ExternalOutput")

    with nc.allow_low_precision(reason="bf16 matmul pipeline"), \
         tile.TileContext(nc) as tc, \
         tc.tile_pool(name="sb", bufs=1) as sb, \
         tc.tile_pool(name="ps", bufs=1, space="PSUM") as ps:

        def mm_ps():
            return ps.tile([128, S], f32, tag="mm", bufs=6, name="mmps")

        def wblock(src, b):
            """One [128, 4096] bf16 weight block = 8 lhsT chunks, one 1MB DMA."""
            w = sb.tile([128, 4096], bf16, tag="wbig", bufs=WBUFS, name="wbig")
            nc.sync.dma_start(w[:], src[:, b * 4096:(b + 1) * 4096])
            return w

        xtiles = [None] * NSEG

        def load_x(p):
            x0 = sb.tile([128, S], bf16, tag="x0", bufs=2, name="x0")
            xr = sb.tile([128, (HK - 1) * S], bf16, tag="xr", bufs=2, name="xr")
            base = p * HK * S
            nc.scalar.dma_start(x0[:], xT[:, base:base + S])
            nc.scalar.dma_start(xr[:], xT[:, base + S:base + HK * S])
            xtiles[p] = (x0, xr)

        def xsl(p, k):
            x0, xr = xtiles[p]
            return x0[:] if k == 0 else xr[:, (k - 1) * S:k * S]

        load_x(0)

        # consts emitted after pass-0 x loads: keeps the scalar DMA ring free
        # so the first in_proj matmul starts as early as possible
        carryall = sb.tile([128, IK * 4], bf16, tag="carryall", name="carryall")
        carryout = sb.tile([128, IK * 4], bf16, tag="carryout", name="carryout")
        nc.scalar.dma_start(carryall[:], haloh[:])
        carry = [carryall[:, i2 * 4:i2 * 4 + 3] for i2 in range(IK)]
        ones_col = sb.tile([128, 1], bf16, tag="ones_col", name="ones_col")
        nc.vector.memset(ones_col[:], 1.0)
        fcb_t = sb.tile([128, HK], f32, tag="fcb", name="fcb")
        nc.scalar.dma_start(fcb_t[:], fcb[:])
        sfc_t = sb.tile([1, H], bf16, tag="sfc", name="sfc")
        nc.scalar.dma_start(sfc_t[:], sfc[:])
        cb_t = sb.tile([128, IK], f32, tag="cb", name="cb")
        nc.scalar.dma_start(cb_t[:], cbcol[:])
        youb_t = sb.tile([128, HK], f32, tag="youb", name="youb")
        nc.scalar.dma_start(youb_t[:], youb[:])

        for p in range(NSEG):
            t0 = p * S
            res_t = [None] * IK
            pst0 = ps.tile([1, S], f32, tag="aux", bufs=2, name="auxps")
            pst1 = ps.tile([1, S], f32, tag="aux", bufs=2, name="auxps")

            # ---- in_proj + conv + gate + interleaved LN stats, per group
            for g in range(G):
                if p == 0 and g == 0:
                    # fast start: 8 independent small tiles so the first
                    # matmul only waits on one 128KB DMA, not a 1MB block
                    w0s = []
                    for k in range(HK):
                        wk = sb.tile([128, 512], bf16, tag="w0s", bufs=8, name="w0s")
                        nc.sync.dma_start(wk[:], win[:, k * 512:(k + 1) * 512])
                        w0s.append(wk)
                    wsl = lambda k, c0: w0s[k][:, c0:c0 + 128]
                else:
                    wg = wblock(win, g)
                    wsl = lambda k, c0: wg[:, k * 512 + c0:k * 512 + c0 + 128]
                cwt = sb.tile([128, 2048], bf16, tag="cw", bufs=2, name="cw")
                nc.scalar.dma_start(cwt[:], cw[:, g * 2048:(g + 1) * 2048])

                hts = []
                for m in range(2):
                    i2 = 2 * g + m
                    ht = sb.tile([128, S + 3], bf16, tag="hT", bufs=3, name="hT")
                    pm = mm_ps()
                    for k in range(HK):
                        nc.tensor.matmul(pm[:], wsl(k, m * 128), xsl(p, k),
                                         start=(k == 0), stop=(k == HK - 1))
                    nc.vector.tensor_copy(ht[:, 3:S + 3], pm[:])
                    nc.vector.tensor_copy(ht[:, 0:3], carry[i2])
                    if p == 0:
                        nc.vector.tensor_copy(carryout[:, i2 * 4:i2 * 4 + 3],
                                              ht[:, S:S + 3])
                        carry[i2] = carryout[:, i2 * 4:i2 * 4 + 3]
                    hts.append(ht)

                gss = []
                for m in range(2):
                    pg = mm_ps()
                    for k in range(HK):
                        nc.tensor.matmul(pg[:], wsl(k, 256 + m * 128), xsl(p, k),
                                         start=(k == 0), stop=(k == HK - 1))
                    gs = sb.tile([128, S], bf16, tag="gsc", bufs=4, name="gsc")
                    nc.scalar.activation(gs[:], pg[:], AF.Silu)
                    gss.append(gs)

                for m in range(2):
                    i2 = 2 * g + m
                    pc = mm_ps()
                    first = True
                    for cc in range(2):
                        for k in range(CK):
                            c0 = cc * 1024 + k * 256 + m * 128
                            nc.tensor.matmul(pc[:], cwt[:, c0:c0 + 128],
                                             hts[cc][:, k:k + S],
                                             start=first, stop=(cc == 1 and k == CK - 1))
                            first = False
                    rs = sb.tile([128, S], bf16, tag=f"res{i2}", name=f"res{i2}")
                    amracc = sb.tile([128, 1], f32, tag="amracc", bufs=4, name="amracc")
                    # rs = (pc + conv_b)*gate, fused; accum_out unused
                    nc.vector.affine_mul_reduce(rs[:], amracc[:], pc[:], gss[m][:],
                                                1.0, cb_t[:, i2:i2 + 1])
                    res_t[i2] = rs
                    # LN stats interleaved with conv so PE never bubbles at LN
                    nc.tensor.matmul(pst0[:], ones_col[:], rs[:],
                                     start=(i2 == 0), stop=(i2 == IK - 1))
                    sq = sb.tile([128, S], bf16, tag="sq", bufs=4, name="sq")
                    nc.scalar.activation(sq[:], rs[:], AF.Square)
                    nc.tensor.matmul(pst1[:], ones_col[:], sq[:],
                                     start=(i2 == 0), stop=(i2 == IK - 1))

            if p + 1 < NSEG:
                load_x(p + 1)

            # ---- layernorm row math (stats psums already accumulated)
            mneg = sb.tile([1, S], bf16, tag="mneg", name="mneg")
            nc.vector.tensor_scalar(mneg[:], pst0[:], -1.0 / I, None, op0=MUL)
            msq = sb.tile([1, S], f32, tag="lnrow", bufs=2, name="msq")
            nc.scalar.activation(msq[:], pst0[:], AF.Square)
            nc.vector.tensor_scalar(msq[:], msq[:], 1.0 / I, None, op0=MUL)
            vrow = sb.tile([1, S], f32, tag="lnrow", bufs=2, name="vrow")
            nc.vector.tensor_tensor(vrow[:], pst1[:], msq[:], op=SUB)
            nc.vector.tensor_scalar(vrow[:], vrow[:], 1.0 / I, EPS, op0=MUL, op1=ADD)
            sd = sb.tile([1, S], f32, tag="lnrow", bufs=2, name="sd")
            nc.scalar.activation(sd[:], vrow[:], AF.Sqrt)
            arow = sb.tile([1, S], f32, tag="arow", name="arow")
            nc.vector.reciprocal(arow[:], sd[:])                    # rstd
            bcA = sb.tile([128, S], f32, tag="bcA", name="bcA")
            nc.gpsimd.partition_broadcast(bcA[:], arow[:], channels=128)

            # ---- MLP c_fc (I -> H): psum = fcw' @ res - S_fc x m  (K=1 matmul
            # adds the mean correction); then scale by rstd and silu.
            m1_t = [None] * HK
            for jb in range(2):
                pms = [mm_ps() for _ in range(4)]
                for kb in range(2):
                    wb = wblock(fcw, jb * 2 + kb)
                    for j2 in range(4):
                        for kk in range(8):
                            nc.tensor.matmul(pms[j2][:],
                                             wb[:, kk * 512 + j2 * 128:kk * 512 + (j2 + 1) * 128],
                                             res_t[kb * 8 + kk][:],
                                             start=(kb == 0 and kk == 0),
                                             stop=False)
                for j2 in range(4):
                    j = jb * 4 + j2
                    nc.tensor.matmul(pms[j2][:], sfc_t[0:1, j * 128:(j + 1) * 128],
                                     mneg[0:1, :], start=False, stop=True)
                    tmp = sb.tile([128, S], f32, tag="fctmp", bufs=4, name="fctmp")
                    nc.vector.tensor_tensor(tmp[:], pms[j2][:], bcA[:], op=MUL)
                    m1 = sb.tile([128, S], bf16, tag=f"m1_{j}", name=f"m1_{j}")
                    nc.scalar.activation(m1[:], tmp[:], AF.Silu,
                                         bias=fcb_t[:, j:j + 1])
                    m1_t[j] = m1

            # ---- MLP c_proj (H -> I) + residual add (c_proj bias folded into
            # the final output bias youb = W_out @ b_cproj)
            oin_t = [None] * IK
            for ib in range(4):
                wb = wblock(cpw, ib)
                for i2 in range(4):
                    i = ib * 4 + i2
                    pm = mm_ps()
                    for kk in range(HK):
                        nc.tensor.matmul(pm[:], wb[:, kk * 512 + i2 * 128:kk * 512 + (i2 + 1) * 128],
                                         m1_t[kk][:], start=(kk == 0), stop=(kk == HK - 1))
                    oi = sb.tile([128, S], bf16, tag=f"oin{i}", name=f"oin{i}")
                    nc.vector.tensor_tensor(oi[:], pm[:], res_t[i][:], op=ADD)
                    oin_t[i] = oi

            # ---- out_proj (I -> H), + youb output bias
            for jb in range(2):
                pms = [mm_ps() for _ in range(4)]
                for kb in range(2):
                    wb = wblock(outw, jb * 2 + kb)
                    for j2 in range(4):
                        for kk in range(8):
                            nc.tensor.matmul(pms[j2][:],
                                             wb[:, kk * 512 + j2 * 128:kk * 512 + (j2 + 1) * 128],
                                             oin_t[kb * 8 + kk][:],
                                             start=(kb == 0 and kk == 0),
                                             stop=(kb == 1 and kk == 7))
                for j2 in range(4):
                    j = jb * 4 + j2
                    yo = sb.tile([128, S], f32, tag="yo", bufs=4, name="yo")
                    nc.vector.tensor_scalar(yo[:], pms[j2][:], youb_t[:, j:j + 1],
                                            None, op0=ADD)
                    nc.scalar.dma_start(yT[j * 128:(j + 1) * 128, t0:t0 + S], yo[:])

    nc.compile()
    return nc


def _pack(inputs):
    import ml_dtypes
    bf = ml_dtypes.bfloat16
    f = lambda name: np.asarray(inputs[name], np.float32)
    hs = np.ascontiguousarray(f("hidden_states"))
    wT = np.ascontiguousarray(f("in_proj_w").T)                 # [H, 2I]
    winp = np.empty((H, 2 * I), np.float32)
    for g in range(G):
        winp[:, g * 512:g * 512 + 256] = wT[:, g * 256:(g + 1) * 256]
        winp[:, g * 512 + 256:(g + 1) * 512] = wT[:, I + g * 256:I + (g + 1) * 256]
    # block layouts: [128, nblocks*4096]; block b holds 8 consecutive lhsT
    # chunks [128, 512] so each phase-block is one contiguous 1MB DMA
    winb = np.ascontiguousarray(
        winp.reshape(HK, 128, G, 512).transpose(1, 2, 0, 3).reshape(128, HK * 2 * I)).astype(bf)
    # layernorm gamma/beta folded into c_fc (exact): silu((hn*g+b) @ W.T + c)
    # = silu(hn @ (W*g).T + (c + W @ b))
    fcw_eff = f("fc_w") * f("ln_g")[None, :]
    fcb_eff = f("fc_b") + f("fc_w") @ f("ln_b")
    sfc_row = np.ascontiguousarray(
        fcw_eff.sum(axis=1, dtype=np.float64).astype(np.float32).reshape(1, H)).astype(bf)
    fcwb = np.ascontiguousarray(
        fcw_eff.T.reshape(2, 8, 128, 2, 512).transpose(2, 3, 0, 1, 4).reshape(128, I * H // 128)).astype(bf)
    cpwb = np.ascontiguousarray(
        f("cproj_w").T.reshape(8, 128, 4, 512).transpose(1, 2, 0, 3).reshape(128, H * I // 128)).astype(bf)
    outwb = np.ascontiguousarray(
        f("out_w").T.reshape(2, 8, 128, 2, 512).transpose(2, 3, 0, 1, 4).reshape(128, I * H // 128)).astype(bf)
    v = f("conv_w").reshape(G, 256, 2, 128, CK)                 # [g, j, cc, i, k]
    cwp = np.ascontiguousarray(v.transpose(3, 0, 2, 4, 1).reshape(128, G * 2 * CK * 256)).astype(bf)
    youb_v = f("out_w") @ f("cproj_b")                          # [H]
    shared = dict(
        win=winb, cw=cwp, fcw=fcwb, cpw=cpwb, outw=outwb,
        cbcol=np.ascontiguousarray(f("conv_b").reshape(IK, 128).T),
        youb=np.ascontiguousarray(youb_v.reshape(HK, 128).T),
        sfc=sfc_row,
        fcb=np.ascontiguousarray(fcb_eff.reshape(HK, 128).T),
    )
    ipw_h = f("in_proj_w")[:I]                                  # [I, H]
    in_maps = []
    for c in range(NCORES):
        b, q = divmod(c, QC)
        own = hs[b, q * T:(q + 1) * T]                          # [T, H]
        prev = (np.zeros((3, H), np.float32) if q == 0
                else hs[b, q * T - 3:q * T])
        # [128, seg, k, t] from own [T, H]
        xTc = np.ascontiguousarray(
            own.reshape(NSEG, S, HK, 128).transpose(3, 0, 2, 1).reshape(128, NSEG * HK * S)).astype(bf)
        hh = np.zeros((IK, 128, 4), np.float32)
        hh[:, :, 0:3] = (ipw_h @ prev.T).reshape(IK, 128, 3)    # halo h columns
        hh = np.ascontiguousarray(hh.transpose(1, 0, 2).reshape(128, IK * 4)).astype(bf)
        in_maps.append(dict(xT=xTc, haloh=hh, **shared))
    return in_maps


def _run(inputs, trace=False):
    from concourse.bass_utils import run_bass_kernel_spmd

    nc = _CACHE.get("nc")
    if nc is None:
        nc = _build()
        _CACHE["nc"] = nc
    in_maps = _pack(inputs)
    try:
        res = run_bass_kernel_spmd(nc, in_maps, core_ids=list(range(NCORES)),
                                   trace=trace)
    except Exception:
        # transient NRT_EXEC_UNIT_UNRECOVERABLE has been observed once after a
        # wedged prior run; one retry has always succeeded
        res = run_bass_kernel_spmd(nc, in_maps, core_ids=list(range(NCORES)),
                                   trace=trace)
    y = np.empty((B, L, H), np.float32)
    for c in range(NCORES):
        b, q = divmod(c, QC)
        y[b, q * T:(q + 1) * T, :] = res.results[c]["yT"].T
    return y, res


def kernel(**inputs) -> np.ndarray:
    y, _ = _run(inputs, trace=False)
    return y


# revision 7
# speedup vs baseline: 1.0816x; 1.0816x over previous
"""Trainium2 Bass kernel for nn_MetaMixer_6717328851330.

Computation (see reference):
    p = x @ W_in.T ; h, gate = split(p) ; gate = silu(gate)
    h = causal_grouped_conv1d(h) + b_conv ; h = h * gate       (residual)
    hn = layernorm_I(h) ; m = silu(hn @ W_fc.T + b_fc) @ W_cp.T + b_cp
    y = (m + residual) @ W_out.T

Strategy: pure data-parallel over the 8192 tokens (B*L), 1024 tokens per
core, no collectives. The causal-conv left halo (3 tokens) is handled by
shipping the previous chunk's last 3 h-columns to each core (host
precomputes them) and carrying the segment boundary on-chip.

On-core layout: activations live as [channel, token] tiles so every matmul
in the chain contracts along SBUF partitions with no transposes anywhere
(host pre-transposes x and all weights). The whole matmul pipeline runs in
bf16 (weights and activations; fp32 PSUM accumulate) — bf16 streams at the
same 1 row/cycle as fp32r but halves LDWEIGHTS and all DMA traffic.
Biases are kept off the PE: conv bias rides the fused DVE
(psum+b)*gate op, the c_proj bias is folded into a final per-channel
output bias (W_out @ b_cproj), and the fc bias rides the silu activation.
Each core processes its 1024 tokens in two 512-token segments so all
intermediates stay resident in SBUF; weights stream from HBM per segment.
"""
import sys

sys.path.insert(0, "/opt/trn_rl_repo")
import numpy as np

NCORES = 8
B, L, H, I, G, CK = 2, 4096, 1024, 2048, 8, 4
T = (B * L) // NCORES          # tokens per core
S = 512                        # token segment (= psum bank free dim)
NSEG = T // S
HK = H // 128                  # 8  k-chunks over hidden
IK = I // 128                  # 16 k-chunks over intermediate
QC = NCORES // B               # seq chunks per batch
EPS = 1e-5
WBUFS = 3                      # streamed weight blocks [128,4096] bf16

_CACHE = {}


def _build():
    import concourse.bacc as bacc
    import concourse.mybir as mybir
    import concourse.tile as tile
    from concourse import bass_isa
    from concourse.alu_op_type import AluOpType

    f32 = mybir.dt.float32
    bf16 = mybir.dt.bfloat16
    AF = mybir.ActivationFunctionType
    MUL, ADD, SUB = AluOpType.mult, AluOpType.add, AluOpType.subtract

    nc = bacc.Bacc(None, target_bir_lowering=False)

    xT = nc.dram_tensor("xT", [128, NSEG * HK * S], bf16, kind="ExternalInput")
    win = nc.dram_tensor("win", [128, HK * 2 * I], bf16, kind="ExternalInput")
    cw = nc.dram_tensor("cw", [128, G * 2 * CK * 256], bf16, kind="ExternalInput")
    fcw = nc.dram_tensor("fcw", [128, I * H // 128], bf16, kind="ExternalInput")
    cpw = nc.dram_tensor("cpw", [128, H * I // 128], bf16, kind="ExternalInput")
    outw = nc.dram_tensor("outw", [128, I * H // 128], bf16, kind="ExternalInput")
    cbcol = nc.dram_tensor("cbcol", [128, IK], f32, kind="ExternalInput")
    youb = nc.dram_tensor("youb", [128, HK], f32, kind="ExternalInput")
    sfc = nc.dram_tensor("sfc", [1, H], bf16, kind="ExternalInput")
    haloh = nc.dram_tensor("haloh", [128, IK * 4], bf16, kind="ExternalInput")
    fcb = nc.dram_tensor("fcb", [128, HK], f32, kind="ExternalInput")
    yT = nc.dram_tensor("yT", [H, T], f32, kind="ExternalOutput")

    with nc.allow_low_precision(reason="bf16 matmul pipeline"), \
         tile.TileContext(nc) as tc, \
         tc.tile_pool(name="sb", bufs=1) as sb, \
         tc.tile_pool(name="ps", bufs=1, space="PSUM") as ps:

        def mm_ps():
            return ps.tile([128, S], f32, tag="mm", bufs=6, name="mmps")

        def wblock(src, b):
            """One [128, 4096] bf16 weight block = 8 lhsT chunks, one 1MB DMA."""
            w = sb.tile([128, 4096], bf16, tag="wbig", bufs=WBUFS, name="wbig")
            nc.sync.dma_start(w[:], src[:, b * 4096:(b + 1) * 4096])
            return w

        xtiles = [None] * NSEG

        def load_x(p):
            x0 = sb.tile([128, S], bf16, tag="x0", bufs=2, name="x0")
            xr = sb.tile([128, (HK - 1) * S], bf16, tag="xr", bufs=2, name="xr")
            base = p * HK * S
            nc.scalar.dma_start(x0[:], xT[:, base:base + S])
            nc.scalar.dma_start(xr[:], xT[:, base + S:base + HK * S])
            xtiles[p] = (x0, xr)

        def xsl(p, k):
            x0, xr = xtiles[p]
            return x0[:] if k == 0 else xr[:, (k - 1) * S:k * S]

        load_x(0)

        # consts emitted after pass-0 x loads: keeps the scalar DMA ring free
        # so the first in_proj matmul starts as early as possible
        carryall = sb.tile([128, IK * 4], bf16, tag="carryall", name="carryall")
        carryout = sb.tile([128, IK * 4], bf16, tag="carryout", name="carryout")
        nc.scalar.dma_start(carryall[:], haloh[:])
        carry = [carryall[:, i2 * 4:i2 * 4 + 3] for i2 in range(IK)]
        fcb_t = sb.tile([128, HK], f32, tag="fcb", name="fcb")
        nc.scalar.dma_start(fcb_t[:], fcb[:])
        sfc_t = sb.tile([1, H], bf16, tag="sfc", name="sfc")
        nc.scalar.dma_start(sfc_t[:], sfc[:])
        cb_t = sb.tile([128, IK], f32, tag="cb", name="cb")
        nc.scalar.dma_start(cb_t[:], cbcol[:])
        youb_t = sb.tile([128, HK], f32, tag="youb", name="youb")
        nc.scalar.dma_start(youb_t[:], youb[:])

        for p in range(NSEG):
            t0 = p * S
            res_t = [None] * IK
            # LN stat accumulators, built on DVE (adds) + Scalar (squares);
            # cross-partition reduction happens once per segment on GpSimd.
            xacc = sb.tile([128, S], f32, tag="xacc", name="xacc")
            sqacc = sb.tile([128, S], f32, tag="sqacc", name="sqacc")
            rs0 = sq0 = None

            # ---- in_proj + conv + gate + interleaved LN stats, per group
            for g in range(G):
                if p == 0 and g == 0:
                    # fast start: 8 independent small tiles so the first
                    # matmul only waits on one 128KB DMA, not a 1MB block
                    w0s = []
                    for k in range(HK):
                        wk = sb.tile([128, 512], bf16, tag="w0s", bufs=8, name="w0s")
                        nc.sync.dma_start(wk[:], win[:, k * 512:(k + 1) * 512])
                        w0s.append(wk)
                    wsl = lambda k, c0: w0s[k][:, c0:c0 + 128]
                else:
                    wg = wblock(win, g)
                    wsl = lambda k, c0: wg[:, k * 512 + c0:k * 512 + c0 + 128]
                cwt = sb.tile([128, 2048], bf16, tag="cw", bufs=2, name="cw")
                nc.scalar.dma_start(cwt[:], cw[:, g * 2048:(g + 1) * 2048])

                hts = []
                for m in range(2):
                    i2 = 2 * g + m
                    ht = sb.tile([128, S + 3], bf16, tag="hT", bufs=3, name="hT")
                    pm = mm_ps()
                    for k in range(HK):
                        nc.tensor.matmul(pm[:], wsl(k, m * 128), xsl(p, k),
                                         start=(k == 0), stop=(k == HK - 1))
                    nc.vector.tensor_copy(ht[:, 3:S + 3], pm[:])
                    nc.vector.tensor_copy(ht[:, 0:3], carry[i2])
                    if p == 0:
                        nc.vector.tensor_copy(carryout[:, i2 * 4:i2 * 4 + 3],
                                              ht[:, S:S + 3])
                        carry[i2] = carryout[:, i2 * 4:i2 * 4 + 3]
                    hts.append(ht)

                gss = []
                for m in range(2):
                    pg = mm_ps()
                    for k in range(HK):
                        nc.tensor.matmul(pg[:], wsl(k, 256 + m * 128), xsl(p, k),
                                         start=(k == 0), stop=(k == HK - 1))
                    gs = sb.tile([128, S], bf16, tag="gsc", bufs=4, name="gsc")
                    nc.scalar.activation(gs[:], pg[:], AF.Silu)
                    gss.append(gs)

                for m in range(2):
                    i2 = 2 * g + m
                    pc = mm_ps()
                    first = True
                    for cc in range(2):
                        for k in range(CK):
                            c0 = cc * 1024 + k * 256 + m * 128
                            nc.tensor.matmul(pc[:], cwt[:, c0:c0 + 128],
                                             hts[cc][:, k:k + S],
                                             start=first, stop=(cc == 1 and k == CK - 1))
                            first = False
                    rs = sb.tile([128, S], bf16, tag=f"res{i2}", name=f"res{i2}")
                    amracc = sb.tile([128, 1], f32, tag="amracc", bufs=4, name="amracc")
                    # rs = (pc + conv_b)*gate, fused; accum_out unused
                    nc.vector.affine_mul_reduce(rs[:], amracc[:], pc[:], gss[m][:],
                                                1.0, cb_t[:, i2:i2 + 1])
                    res_t[i2] = rs
                    sq = sb.tile([128, S], bf16, tag="sq", bufs=4, name="sq")
                    nc.scalar.activation(sq[:], rs[:], AF.Square)
                    if i2 == 0:
                        rs0, sq0 = rs, sq
                    elif i2 == 1:
                        nc.vector.tensor_tensor(xacc[:], rs0[:], rs[:], op=ADD)
                        nc.vector.tensor_tensor(sqacc[:], sq0[:], sq[:], op=ADD)
                    else:
                        nc.vector.tensor_tensor(xacc[:], xacc[:], rs[:], op=ADD)
                        nc.vector.tensor_tensor(sqacc[:], sqacc[:], sq[:], op=ADD)

            if p + 1 < NSEG:
                load_x(p + 1)

            # ---- cross-partition stat reduction + layernorm row math
            xred = sb.tile([128, S], f32, tag="xred", name="xred")
            nc.gpsimd.partition_all_reduce(xred[:], xacc[:], 128,
                                           bass_isa.ReduceOp.add)
            sqred = sb.tile([128, S], f32, tag="sqred", name="sqred")
            nc.gpsimd.partition_all_reduce(sqred[:], sqacc[:], 128,
                                           bass_isa.ReduceOp.add)
            mneg = sb.tile([1, S], bf16, tag="mneg", name="mneg")
            nc.vector.tensor_scalar(mneg[:], xred[0:1, :], -1.0 / I, None, op0=MUL)
            msq = sb.tile([1, S], f32, tag="lnrow", bufs=2, name="msq")
            nc.scalar.activation(msq[:], xred[0:1, :], AF.Square)
            nc.vector.tensor_scalar(msq[:], msq[:], 1.0 / I, None, op0=MUL)
            vrow = sb.tile([1, S], f32, tag="lnrow", bufs=2, name="vrow")
            nc.vector.tensor_tensor(vrow[:], sqred[0:1, :], msq[:], op=SUB)
            nc.vector.tensor_scalar(vrow[:], vrow[:], 1.0 / I, EPS, op0=MUL, op1=ADD)
            sd = sb.tile([1, S], f32, tag="lnrow", bufs=2, name="sd")
            nc.scalar.activation(sd[:], vrow[:], AF.Sqrt)
            arow = sb.tile([1, S], f32, tag="arow", name="arow")
            nc.vector.reciprocal(arow[:], sd[:])                    # rstd
            bcA = sb.tile([128, S], f32, tag="bcA", name="bcA")
            nc.gpsimd.partition_broadcast(bcA[:], arow[:], channels=128)

            # ---- MLP c_fc (I -> H): psum = fcw' @ res - S_fc x m  (K=1 matmul
            # adds the mean correction); then scale by rstd and silu.
            m1_t = [None] * HK
            for jb in range(2):
                pms = [mm_ps() for _ in range(4)]
                for kb in range(2):
                    wb = wblock(fcw, jb * 2 + kb)
                    for j2 in range(4):
                        for kk in range(8):
                            nc.tensor.matmul(pms[j2][:],
                                             wb[:, kk * 512 + j2 * 128:kk * 512 + (j2 + 1) * 128],
                                             res_t[kb * 8 + kk][:],
                                             start=(kb == 0 and kk == 0),
                                             stop=False)
                for j2 in range(4):
                    j = jb * 4 + j2
                    nc.tensor.matmul(pms[j2][:], sfc_t[0:1, j * 128:(j + 1) * 128],
                                     mneg[0:1, :], start=False, stop=True)
                    tmp = sb.tile([128, S], f32, tag="fctmp", bufs=4, name="fctmp")
                    nc.vector.tensor_tensor(tmp[:], pms[j2][:], bcA[:], op=MUL)
                    m1 = sb.tile([128, S], bf16, tag=f"m1_{j}", name=f"m1_{j}")
                    nc.scalar.activation(m1[:], tmp[:], AF.Silu,
                                         bias=fcb_t[:, j:j + 1])
                    m1_t[j] = m1

            # ---- MLP c_proj (H -> I) + residual add (c_proj bias folded into
            # the final output bias youb = W_out @ b_cproj)
            oin_t = [None] * IK
            for ib in range(4):
                wb = wblock(cpw, ib)
                for i2 in range(4):
                    i = ib * 4 + i2
                    pm = mm_ps()
                    for kk in range(HK):
                        nc.tensor.matmul(pm[:], wb[:, kk * 512 + i2 * 128:kk * 512 + (i2 + 1) * 128],
                                         m1_t[kk][:], start=(kk == 0), stop=(kk == HK - 1))
                    oi = sb.tile([128, S], bf16, tag=f"oin{i}", name=f"oin{i}")
                    nc.vector.tensor_tensor(oi[:], pm[:], res_t[i][:], op=ADD)
                    oin_t[i] = oi

            # ---- out_proj (I -> H), + youb output bias
            for jb in range(2):
                pms = [mm_ps() for _ in range(4)]
                for kb in range(2):
                    wb = wblock(outw, jb * 2 + kb)
                    for j2 in range(4):
                        for kk in range(8):
                            nc.tensor.matmul(pms[j2][:],
                                             wb[:, kk * 512 + j2 * 128:kk * 512 + (j2 + 1) * 128],
                                             oin_t[kb * 8 + kk][:],
                                             start=(kb == 0 and kk == 0),
                                             stop=(kb == 1 and kk == 7))
                for j2 in range(4):
                    j = jb * 4 + j2
                    yo = sb.tile([128, S], f32, tag="yo", bufs=4, name="yo")
                    nc.vector.tensor_scalar(yo[:], pms[j2][:], youb_t[:, j:j + 1],
                                            None, op0=ADD)
                    nc.scalar.dma_start(yT[j * 128:(j + 1) * 128, t0:t0 + S], yo[:])

    nc.compile()
    return nc


def _pack(inputs):
    import ml_dtypes
    bf = ml_dtypes.bfloat16
    f = lambda name: np.asarray(inputs[name], np.float32)
    hs = np.ascontiguousarray(f("hidden_states"))
    wT = np.ascontiguousarray(f("in_proj_w").T)                 # [H, 2I]
    winp = np.empty((H, 2 * I), np.float32)
    for g in range(G):
        winp[:, g * 512:g * 512 + 256] = wT[:, g * 256:(g + 1) * 256]
        winp[:, g * 512 + 256:(g + 1) * 512] = wT[:, I + g * 256:I + (g + 1) * 256]
    # block layouts: [128, nblocks*4096]; block b holds 8 consecutive lhsT
    # chunks [128, 512] so each phase-block is one contiguous 1MB DMA
    winb = np.ascontiguousarray(
        winp.reshape(HK, 128, G, 512).transpose(1, 2, 0, 3).reshape(128, HK * 2 * I)).astype(bf)
    # layernorm gamma/beta folded into c_fc (exact): silu((hn*g+b) @ W.T + c)
    # = silu(hn @ (W*g).T + (c + W @ b))
    fcw_eff = f("fc_w") * f("ln_g")[None, :]
    fcb_eff = f("fc_b") + f("fc_w") @ f("ln_b")
    sfc_row = np.ascontiguousarray(
        fcw_eff.sum(axis=1, dtype=np.float64).astype(np.float32).reshape(1, H)).astype(bf)
    fcwb = np.ascontiguousarray(
        fcw_eff.T.reshape(2, 8, 128, 2, 512).transpose(2, 3, 0, 1, 4).reshape(128, I * H // 128)).astype(bf)
    cpwb = np.ascontiguousarray(
        f("cproj_w").T.reshape(8, 128, 4, 512).transpose(1, 2, 0, 3).reshape(128, H * I // 128)).astype(bf)
    outwb = np.ascontiguousarray(
        f("out_w").T.reshape(2, 8, 128, 2, 512).transpose(2, 3, 0, 1, 4).reshape(128, I * H // 128)).astype(bf)
    v = f("conv_w").reshape(G, 256, 2, 128, CK)                 # [g, j, cc, i, k]
    cwp = np.ascontiguousarray(v.transpose(3, 0, 2, 4, 1).reshape(128, G * 2 * CK * 256)).astype(bf)
    youb_v = f("out_w") @ f("cproj_b")                          # [H]
    shared = dict(
        win=winb, cw=cwp, fcw=fcwb, cpw=cpwb, outw=outwb,
        cbcol=np.ascontiguousarray(f("conv_b").reshape(IK, 128).T),
        youb=np.ascontiguousarray(youb_v.reshape(HK, 128).T),
        sfc=sfc_row,
        fcb=np.ascontiguousarray(fcb_eff.reshape(HK, 128).T),
    )
    ipw_h = f("in_proj_w")[:I]                                  # [I, H]
    in_maps = []
    for c in range(NCORES):
        b, q = divmod(c, QC)
        own = hs[b, q * T:(q + 1) * T]                          # [T, H]
        prev = (np.zeros((3, H), np.float32) if q == 0
                else hs[b, q * T - 3:q * T])
        # [128, seg, k, t] from own [T, H]
        xTc = np.ascontiguousarray(
            own.reshape(NSEG, S, HK, 128).transpose(3, 0, 2, 1).reshape(128, NSEG * HK * S)).astype(bf)
        hh = np.zeros((IK, 128, 4), np.float32)
        hh[:, :, 0:3] = (ipw_h @ prev.T).reshape(IK, 128, 3)    # halo h columns
        hh = np.ascontiguousarray(hh.transpose(1, 0, 2).reshape(128, IK * 4)).astype(bf)
        in_maps.append(dict(xT=xTc, haloh=hh, **shared))
    return in_maps


def _run(inputs, trace=False):
    from concourse.bass_utils import run_bass_kernel_spmd

    nc = _CACHE.get("nc")
    if nc is None:
        nc = _build()
        _CACHE["nc"] = nc
    in_maps = _pack(inputs)
    try:
        res = run_bass_kernel_spmd(nc, in_maps, core_ids=list(range(NCORES)),
                                   trace=trace)
    except Exception:
        # transient NRT_EXEC_UNIT_UNRECOVERABLE has been observed once after a
        # wedged prior run; one retry has always succeeded
        res = run_bass_kernel_spmd(nc, in_maps, core_ids=list(range(NCORES)),
                                   trace=trace)
    y = np.empty((B, L, H), np.float32)
    for c in range(NCORES):
        b, q = divmod(c, QC)
        y[b, q * T:(q + 1) * T, :] = res.results[c]["yT"].T
    return y, res


def kernel(**inputs) -> np.ndarray:
    y, _ = _run(inputs, trace=False)
    return y
